# revision 58
# baseline (speedup 1.0000x reference)
"""Trainium2 Bass kernel for nn_AudioEncoder (4-layer conformer-style encoder,
TransformerXL relative-position attention), distributed over 8 NeuronCores.

Sharding: DP=2 over batch (cores 0-3 <-> batch 0, cores 4-7 <-> batch 1),
TP=4 within each quad over heads (4 heads/core) and the FFN dim (1024/core).
Two AllReduces per layer (attention output proj, FFN second matmul) over each
4-core replica group, in bf16; each AllReduce is split into token-halves that
pipeline against the other half's compute so the TensorEngine never idles
across a collective.

On-device layout is feature-major: activations live as [C, tokens] so every
matmul contracts over the partition axis. LayerNorm statistics (reductions
over C = partitions) are computed with [128,128]-ones matmuls that land the
sums pre-broadcast on all partitions (single-partition DVE ops are ~9ns/elem
and must be avoided).
"""

import contextlib

import numpy as np
import ml_dtypes

import concourse.bass as bass
import concourse.mybir as mybir
import concourse.tile as tile
from concourse import bacc
from concourse.bass_utils import run_bass_kernel_spmd

BF16 = mybir.dt.bfloat16
F32 = mybir.dt.float32
AF = mybir.ActivationFunctionType
ALU = mybir.AluOpType
NEG_MASK = -30000.0


class Cfg:
    def __init__(self, L=4, B=2, N=1024, C=1024, H=16, D=64, F=4096,
                 TP=4, eps=1e-5, nsp=None):
        self.L, self.B, self.N, self.C, self.H, self.D, self.F = \
            L, B, N, C, H, D, F
        self.TP = TP
        self.DP = 2
        self.ncores = self.TP * self.DP
        self.eps = eps
        self.HL = H // TP            # heads per core
        self.DH = self.HL * D        # head dims per core
        self.FL = F // TP            # ffn dims per core
        self.CCH = C // 128          # C chunks
        self.HB = self.DH // 128     # head blocks (2 heads each)
        self.FCH = self.FL // 128    # ffn chunks
        self.KCH = N // 128          # token chunks
        self.NSP = nsp or min(512, N)  # matmul free-dim / token group size
        self.NH = N // self.NSP      # number of token groups
        self.scale = D ** -0.5
        assert self.HL % 2 == 0 and D == 64 and C % 128 == 0
        assert self.FL % 128 == 0 and N % 128 == 0


def build_nc(cfg, apply_gb=True, use_vrow=True, use_ars=False):
    """Builds the per-core Bass program (SPMD: same graph on all 8 cores)."""
    nc = bacc.Bacc("TRN2", target_bir_lowering=False, debug=False,
                   num_devices=cfg.ncores)
    L, N, CCH, HB, FCH, KCH, NSP = (cfg.L, cfg.N, cfg.CCH, cfg.HB, cfg.FCH,
                                    cfg.KCH, cfg.NSP)
    NHS = [slice(i * NSP, (i + 1) * NSP) for i in range(cfg.NH)]
    G = cfg.NH
    RG = [[g * cfg.TP + i for i in range(cfg.TP)] for g in range(cfg.DP)]

    # ---- DRAM I/O ----
    d = {}
    d["xT"] = nc.dram_tensor("xT", [cfg.C, N], BF16, kind="ExternalInput").ap()
    d["posT"] = nc.dram_tensor("posT", [cfg.C, N], BF16, kind="ExternalInput").ap()
    for nm, shp in [("wq", [L, cfg.C, cfg.DH]), ("wk", [L, cfg.C, cfg.DH]),
                    ("wv", [L, cfg.C, cfg.DH]), ("wp", [L, cfg.C, cfg.DH]),
                    ("wo", [L, cfg.DH, cfg.C]), ("w1", [L, cfg.C, cfg.FL]),
                    ("w2", [L, cfg.FL, cfg.C])]:
        d[nm] = nc.dram_tensor(nm, shp, BF16, kind="ExternalInput").ap()
    for nm, shp in [("ub", [L, 128, HB]), ("vb", [L, 128, HB]),
                    ("kb", [L, 128, HB]),
                    ("bo", [L, 128, CCH]), ("bf2", [L, 128, CCH]),
                    ("bf1", [L, 128, FCH]),
                    ("g1", [L, 128, CCH]), ("b1", [L, 128, CCH]),
                    ("g2", [L, 128, CCH]), ("b2", [L, 128, CCH]),
                    ("gf", [128, CCH]), ("bf", [128, CCH]),
                    ("maskb", [128, KCH])]:
        d[nm] = nc.dram_tensor(nm, shp, F32, kind="ExternalInput").ap()
    d["vrow"] = nc.dram_tensor("vrow", [L, 1, cfg.DH], BF16,
                               kind="ExternalInput").ap()
    out_d = nc.dram_tensor("out", [128, CCH, N], F32, kind="ExternalOutput").ap()

    ones_bc_d = nc.inline_tensor(np.ones((128, 128), ml_dtypes.bfloat16), "ones_bc")
    ones64_d = nc.inline_tensor(np.ones((128, 64), ml_dtypes.bfloat16), "ones64")
    ones_row_d = nc.inline_tensor(np.ones((1, 128), ml_dtypes.bfloat16), "ones_row")

    with tile.TileContext(nc) as tc:
        ctx = contextlib.ExitStack()
        with ctx:
            consts = ctx.enter_context(tc.tile_pool(name="consts", bufs=1))
            resid = ctx.enter_context(tc.tile_pool(name="resid", bufs=1))
            wpool = ctx.enter_context(tc.tile_pool(name="wpool", bufs=1))
            work = ctx.enter_context(tc.tile_pool(name="work", bufs=1))
            lnw = ctx.enter_context(tc.tile_pool(name="lnw", bufs=2))
            tmp = ctx.enter_context(tc.tile_pool(name="tmp", bufs=6))
            strm = ctx.enter_context(tc.tile_pool(name="strm", bufs=3))
            psum = ctx.enter_context(tc.tile_pool(name="psum", bufs=2, space="PSUM"))
            sumsp = ctx.enter_context(tc.tile_pool(name="sumsp", bufs=2, space="PSUM"))
            dram = ctx.enter_context(tc.tile_pool(name="dram", bufs=2, space="DRAM"))

            # ---- constants / resident tensors ----
            ones_bc = consts.tile([128, 128], BF16)
            nc.sync.dma_start(ones_bc, ones_bc_d.ap())
            ones_row = consts.tile([1, 128], BF16)
            nc.sync.dma_start(ones_row, ones_row_d.ap())
            ones64 = consts.tile([128, 64], BF16)
            nc.sync.dma_start(ones64, ones64_d.ap())
            gf_t = consts.tile([128, CCH], F32)
            nc.sync.dma_start(gf_t, d["gf"])
            bfin_t = consts.tile([128, CCH], F32)
            nc.sync.dma_start(bfin_t, d["bf"])
            maskb_t = consts.tile([128, KCH], F32)
            nc.sync.dma_start(maskb_t, d["maskb"])
            eps_t = consts.tile([128, 1], F32)
            nc.vector.memset(eps_t, float(cfg.eps))
            zero_t = consts.tile([128, 1], F32)
            nc.vector.memset(zero_t, 0.0)
            posT = consts.tile([128, CCH, N], BF16)
            nc.sync.dma_start(posT, d["posT"].rearrange("(k p) n -> p k n", p=128))

            hT = resid.tile([128, CCH, N], BF16)
            xr = d["xT"].rearrange("(k p) n -> p k n", p=128)
            for i in range(cfg.NH):   # per-group loads so LN1(g0) starts early
                nhg = slice(i * NSP, (i + 1) * NSP)
                nc.sync.dma_start(hT[:, :, nhg], xr[:, :, nhg])

            def _load(l, mats, smalls, vrow=False):
                w = {}
                for nm, shp in mats:
                    w[nm] = wpool.tile(shp, BF16, tag=nm, name=nm)
                    nc.sync.dma_start(
                        w[nm], d[nm][l].rearrange("(k p) m -> p k m", p=128))
                for nm, width in smalls:
                    w[nm] = wpool.tile([128, width], F32, tag=nm,
                                       name=f"sm_{nm}")
                    nc.sync.dma_start(w[nm], d[nm][l])
                if vrow and use_vrow:
                    w["vrow"] = wpool.tile([1, cfg.DH], BF16, tag="vrow",
                                           name="vrow")
                    nc.sync.dma_start(w["vrow"], d["vrow"][l])
                return w

            def load_w_qkvp(l):
                # weights consumed by LN1(l)/qkv(l) (emitted in layer l-1's
                # tail) and p-proj(l) — safe to load during layer l-1.
                return _load(
                    l,
                    [("wq", [128, CCH, cfg.DH]), ("wk", [128, CCH, cfg.DH]),
                     ("wv", [128, CCH, cfg.DH]), ("wp", [128, CCH, cfg.DH])],
                    [("ub", HB), ("vb", HB), ("kb", HB), ("g1", CCH),
                     ("b1", CCH)],
                    vrow=True)

            def load_w_rest(l):
                return _load(
                    l,
                    [("wo", [128, HB, cfg.C]), ("w1", [128, CCH, cfg.FL]),
                     ("w2", [128, FCH, cfg.C])],
                    [("bo", CCH), ("bf2", CCH), ("bf1", FCH), ("g2", CCH),
                     ("b2", CCH)])

            # ---------- per-token-group layernorm (feature-major) ----------
            def ln_stats(src, nhg):
                """LN stats over C (partitions) for token group nhg. The
                [128,128] ones lhsT lands sums pre-broadcast on all
                partitions. Returns bf16 (a, b) rows: hn = x*a + b."""
                s0 = sumsp.tile([128, NSP], F32, tag="asum", bufs=2, name="s0")
                s1 = sumsp.tile([128, NSP], F32, tag="asum", bufs=2, name="s1")
                for c in range(CCH):
                    xsq = tmp.tile([128, NSP], BF16, tag="tmp")
                    nc.vector.tensor_mul(xsq, src[:, c, nhg], src[:, c, nhg])
                    nc.tensor.matmul(s0, ones_bc, src[:, c, nhg],
                                     start=(c == 0), stop=(c == CCH - 1))
                    nc.tensor.matmul(s1, ones_bc, xsq,
                                     start=(c == 0), stop=(c == CCH - 1))
                # msq = (s0/C)^2 ; var = s1/C - msq ; a = 1/sqrt(var+eps)
                t_sb = lnw.tile([128, NSP], F32, tag="t_sb")
                nc.scalar.activation(t_sb, s0, AF.Square, bias=zero_t,
                                     scale=1.0 / cfg.C)
                nc.vector.scalar_tensor_tensor(
                    t_sb, in0=s1, scalar=1.0 / cfg.C, in1=t_sb,
                    op0=ALU.mult, op1=ALU.subtract)
                a_sb = lnw.tile([128, NSP], F32, tag="a_sb")
                if use_ars:
                    nc.scalar.activation(a_sb, t_sb, AF.Abs_reciprocal_sqrt,
                                         bias=eps_t)
                else:
                    nc.scalar.activation(t_sb, t_sb, AF.Sqrt, bias=eps_t)
                    nc.vector.reciprocal_approx_fast(a_sb, t_sb)
                # bf16 scale/shift rows: all-bf16 SBUF operands run the DVE
                # in 4x mode for the apply passes; b = -(s0/C)*a
                a_bf = lnw.tile([128, NSP], BF16, tag="a_bf")
                nc.vector.tensor_copy(a_bf, a_sb)
                b_bf = lnw.tile([128, NSP], BF16, tag="b_bf")
                nc.vector.scalar_tensor_tensor(
                    b_bf, in0=s0, scalar=-1.0 / cfg.C, in1=a_sb,
                    op0=ALU.mult, op1=ALU.mult)
                return a_bf, b_bf

            def ln_apply(src, nhg, ab, g_col, beta_col, dst=None,
                         out_f32=False):
                a_bf, b_bf = ab
                for c in range(CCH):
                    t1 = tmp.tile([128, NSP], BF16, tag="tmp")
                    nc.vector.tensor_mul(t1, src[:, c, nhg], a_bf)
                    if out_f32:
                        oc = strm.tile([128, NSP], F32, tag="outc")
                        nc.vector.tensor_add(oc, t1, b_bf)
                        if apply_gb:
                            nc.vector.tensor_scalar(
                                oc, oc, g_col[:, c:c + 1], beta_col[:, c:c + 1],
                                op0=ALU.mult, op1=ALU.add)
                        nc.sync.dma_start(out_d[:, c, nhg], oc)
                    elif apply_gb:
                        t2 = tmp.tile([128, NSP], BF16, tag="tmp")
                        nc.vector.tensor_add(t2, t1, b_bf)
                        nc.vector.tensor_scalar(
                            dst[:, c, nhg], t2, g_col[:, c:c + 1],
                            beta_col[:, c:c + 1], op0=ALU.mult, op1=ALU.add)
                    else:
                        nc.vector.tensor_add(dst[:, c, nhg], t1, b_bf)

            # ---------- q/k/v projections for one token group ----------
            def emit_qkv(w, hn, qTu, qTv, kT, v_t, g):
                nhg = NHS[g]
                for blk in range(HB):
                    ps = psum.tile([128, NSP], F32, tag="mm")
                    for c in range(CCH):
                        nc.tensor.matmul(
                            ps, w["wq"][:, c, blk * 128:(blk + 1) * 128],
                            hn[:, c, nhg], start=(c == 0), stop=(c == CCH - 1))
                    nc.vector.tensor_scalar(qTu[:, blk, nhg], ps,
                                            w["ub"][:, blk:blk + 1], None,
                                            op0=ALU.add)
                    nc.vector.tensor_scalar(qTv[:, blk, nhg], ps,
                                            w["vb"][:, blk:blk + 1], None,
                                            op0=ALU.add)
                    ps = psum.tile([128, NSP], F32, tag="mm")
                    for c in range(CCH):
                        nc.tensor.matmul(
                            ps, w["wk"][:, c, blk * 128:(blk + 1) * 128],
                            hn[:, c, nhg], start=(c == 0), stop=(c == CCH - 1))
                    nc.vector.tensor_scalar(kT[:, blk, nhg], ps,
                                            w["kb"][:, blk:blk + 1], None,
                                            op0=ALU.add)
                # v: token-major
                for kc in range(g * KCH // G, (g + 1) * KCH // G):
                    ps = psum.tile([128, cfg.DH], F32, tag="mm")
                    for c in range(CCH):
                        nc.tensor.matmul(
                            ps, hn[:, c, kc * 128:(kc + 1) * 128], w["wv"][:, c, :],
                            start=(c == 0),
                            stop=(c == CCH - 1) and not use_vrow)
                    if use_vrow:
                        nc.tensor.matmul(ps, ones_row, w["vrow"],
                                         start=False, stop=True)
                    nc.vector.tensor_copy(v_t[:, kc, :], ps)

            def emit_pproj(wA, l):
                pT = work.tile([128, HB, N], BF16, tag="pT", name=f"pT{l}",
                               bufs=2)
                for blk in range(HB):
                    for nh in NHS:
                        ps = psum.tile([128, NSP], F32, tag="mm")
                        for c in range(CCH):
                            nc.tensor.matmul(
                                ps, wA["wp"][:, c, blk * 128:(blk + 1) * 128],
                                posT[:, c, nh],
                                start=(c == 0), stop=(c == CCH - 1))
                        nc.vector.tensor_copy(pT[:, blk, nh], ps)
                return pT

            # ================= program =================
            wA = load_w_qkvp(0)
            hn = work.tile([128, CCH, N], BF16, tag="hn", name="hn0", bufs=2)
            qTu = work.tile([128, HB, N], BF16, tag="qTu", name="qTu0")
            qTv = work.tile([128, HB, N], BF16, tag="qTv", name="qTv0")
            kT = work.tile([128, HB, N], BF16, tag="kT", name="kT0")
            v_t = work.tile([128, KCH, cfg.DH], BF16, tag="v_t", name="v0")
            for g in range(G):
                ab = ln_stats(hT, NHS[g])
                ln_apply(hT, NHS[g], ab, wA["g1"], wA["b1"], dst=hn)
                emit_qkv(wA, hn, qTu, qTv, kT, v_t, g)
            pT = emit_pproj(wA, 0)

            def emit_resid(g, ar_out, bias_col):
                nhg = NHS[g]
                for cc in range(CCH):
                    art = strm.tile([128, NSP], BF16, tag="art")
                    nc.sync.dma_start(art, ar_out[:, cc, :])
                    nc.vector.scalar_tensor_tensor(
                        hT[:, cc, nhg], in0=art, scalar=bias_col[:, cc:cc + 1],
                        in1=hT[:, cc, nhg], op0=ALU.add, op1=ALU.add)

            def emit_allreduce(src_mm, g, name):
                """src_mm(cc) emits accumulating matmuls into a psum tile;
                copies to bf16, DMAs to a DRAM bounce, AllReduces."""
                ai = dram.tile([128, CCH, NSP], BF16, tag="ar_in",
                               name=f"{name}i")
                ao = dram.tile([128, CCH, NSP], BF16, tag="ar_out",
                               name=f"{name}o")
                for cc in range(CCH):
                    ps = psum.tile([128, NSP], F32, tag="mm")
                    src_mm(ps, cc)
                    arc = strm.tile([128, NSP], BF16, tag="arc")
                    nc.vector.tensor_copy(arc, ps)
                    nc.sync.dma_start(ai[:, cc, :], arc)
                nc.gpsimd.collective_compute(
                    "AllReduce", ALU.add, replica_groups=RG,
                    ins=[ai[:].opt()], outs=[ao[:].opt()])
                return ao

            for l in range(L):
                w = {**wA, **load_w_rest(l)}
                last = l == L - 1

                # -- attention + output projection + AR1, token-group-outer:
                # group g0's AllReduce launches while group g1's attention
                # still computes, so the collective hides behind the PE.
                oTn = work.tile([128, HB, N], BF16, tag="oTn", name=f"oTn{l}")
                ar1_out = []
                ab2 = {}
                for ig, nh in enumerate(NHS):
                    po = {}
                    sep = {}
                    for blk in range(HB):
                        po[blk] = psum.tile([128, NSP], F32, tag="po",
                                            bufs=2, name=f"po{blk}")
                        sep[blk] = sumsp.tile([128, NSP], F32, tag="sep",
                                              bufs=2, name=f"sep{blk}")
                    for kc in range(KCH):
                        kcs = slice(kc * 128, (kc + 1) * 128)
                        for blk in range(HB):
                            es = {}
                            # score pairs share PE row groups (K=64, rows 0/64)
                            for hr in (0, 1):
                                psl = slice(hr * 64, hr * 64 + 64)
                                s = psum.tile([128, NSP], F32, tag="mm",
                                              name=f"s{hr}")
                                nc.tensor.matmul(s, kT[psl, blk, kcs],
                                                 qTu[psl, blk, nh],
                                                 start=True, stop=False)
                                nc.tensor.matmul(s, pT[psl, blk, kcs],
                                                 qTv[psl, blk, nh],
                                                 start=False, stop=True)
                                es[hr] = s
                            ee = {}
                            for hr in (0, 1):
                                e = tmp.tile([128, NSP], BF16, tag="tmp",
                                             name=f"e{hr}")
                                nc.scalar.activation(e, es[hr], AF.Exp,
                                                     bias=maskb_t[:, kc:kc + 1],
                                                     scale=float(cfg.scale))
                                ee[hr] = e
                            # attn@v and exp-sum pairs share PE column groups
                            for hr in (0, 1):
                                h = blk * 2 + hr
                                psl = slice(hr * 64, hr * 64 + 64)
                                nc.tensor.matmul(
                                    po[blk][psl, :],
                                    v_t[:, kc, h * 64:(h + 1) * 64],
                                    ee[hr], start=(kc == 0),
                                    stop=(kc == KCH - 1), skip_group_check=True)
                            for hr in (0, 1):
                                psl = slice(hr * 64, hr * 64 + 64)
                                nc.tensor.matmul(
                                    sep[blk][psl, :], ones64, ee[hr],
                                    start=(kc == 0), stop=(kc == KCH - 1),
                                    skip_group_check=True)
                    for blk in range(HB):
                        sesb = lnw.tile([128, NSP], F32, tag="sesb")
                        nc.vector.tensor_copy(sesb, sep[blk])
                        rcp = lnw.tile([128, NSP], F32, tag="rcpf")
                        nc.vector.reciprocal_approx_fast(rcp, sesb)
                        nc.vector.tensor_mul(oTn[:, blk, nh], po[blk], rcp)

                    def oproj_mm(ps, cc, nh=nh):
                        for db in range(HB):
                            nc.tensor.matmul(
                                ps, w["wo"][:, db, cc * 128:(cc + 1) * 128],
                                oTn[:, db, nh],
                                start=(db == 0), stop=(db == HB - 1))
                    ar1_out.append(emit_allreduce(oproj_mm, ig, f"a1_{ig}"))
                    if ig == 0 and not last:
                        # weight DMAs issue early (attention-g1 already
                        # covers AR1(g0); the p-proj filler goes after FFN)
                        wA_next = load_w_qkvp(l + 1)

                # -- FFN pipeline: group g's FFN runs under the other
                # group's AllReduces; resid+stats hoisted to overlap PE work
                hn2 = work.tile([128, CCH, N], BF16, tag="hn",
                                name=f"hn2_{l}", bufs=2)
                fT = work.tile([128, FCH, N], BF16, tag="fT", name=f"fT{l}")
                ar2_out = {}

                def emit_ffn(g):
                    nhg = NHS[g]
                    ln_apply(hT, nhg, ab2[g], w["g2"], w["b2"], dst=hn2)
                    for fc in range(FCH):
                        ps = psum.tile([128, NSP], F32, tag="mm")
                        for c in range(CCH):
                            nc.tensor.matmul(
                                ps, w["w1"][:, c, fc * 128:(fc + 1) * 128],
                                hn2[:, c, nhg],
                                start=(c == 0), stop=(c == CCH - 1))
                        nc.scalar.activation(fT[:, fc, nhg], ps, AF.Relu,
                                             bias=w["bf1"][:, fc:fc + 1])

                    def ffn2_mm(ps, cc, nhg=nhg):
                        for fc in range(FCH):
                            nc.tensor.matmul(
                                ps, w["w2"][:, fc, cc * 128:(cc + 1) * 128],
                                fT[:, fc, nhg],
                                start=(fc == 0), stop=(fc == FCH - 1))
                    ar2_out[g] = emit_allreduce(ffn2_mm, g, f"a2_{g}")

                for g in range(G):
                    emit_resid(g, ar1_out[g], w["bo"])
                    ab2[g] = ln_stats(hT, NHS[g])
                    emit_ffn(g)
                if not last:
                    # PE filler while the last AR2 is in flight
                    pT_next = emit_pproj(wA_next, l + 1)

                # -- tail: residual2 + (next-layer LN1+qkv | final LN) --
                if not last:
                    hn = work.tile([128, CCH, N], BF16, tag="hn",
                                   name=f"hn{l + 1}", bufs=2)
                    qTu = work.tile([128, HB, N], BF16, tag="qTu",
                                    name=f"qTu{l + 1}")
                    qTv = work.tile([128, HB, N], BF16, tag="qTv",
                                    name=f"qTv{l + 1}")
                    kT = work.tile([128, HB, N], BF16, tag="kT",
                                   name=f"kT{l + 1}")
                    v_t = work.tile([128, KCH, cfg.DH], BF16, tag="v_t",
                                    name=f"v{l + 1}")
                for g in range(G):
                    emit_resid(g, ar2_out[g], w["bf2"])
                    ab = ln_stats(hT, NHS[g])
                    if last:
                        ln_apply(hT, NHS[g], ab, gf_t, bfin_t, out_f32=True)
                    else:
                        ln_apply(hT, NHS[g], ab, wA_next["g1"],
                                 wA_next["b1"], dst=hn)
                        emit_qkv(wA_next, hn, qTu, qTv, kT, v_t, g)
                if not last:
                    wA = wA_next
                    pT = pT_next

    nc.compile()
    return nc


# ---------------------------------------------------------------------------
# host side
# ---------------------------------------------------------------------------

def _bf16(a):
    return np.ascontiguousarray(np.asarray(a, np.float32).astype(ml_dtypes.bfloat16))


def _cvec(a, L, nch):
    # [L, nch*128] -> [L, 128, nch]
    return np.ascontiguousarray(
        np.asarray(a, np.float32).reshape(L, nch, 128).transpose(0, 2, 1))


def make_in_maps(cfg, inputs):
    """Shard + lay out full inputs for the 8 cores."""
    L, TP = cfg.L, cfg.TP
    x = np.asarray(inputs["x"], np.float32)
    mask = np.asarray(inputs["attention_mask"])
    pos = np.asarray(inputs["pos_embeds"], np.float32)[0]      # [N, C]
    posT = _bf16(pos.T)
    in_maps = []
    for core in range(cfg.ncores):
        tp, b = core % TP, core // TP
        dsl = slice(tp * cfg.DH, (tp + 1) * cfg.DH)
        hsl = slice(tp * cfg.HL, (tp + 1) * cfg.HL)
        fsl = slice(tp * cfg.FL, (tp + 1) * cfg.FL)
        m = {}
        m["xT"] = _bf16(x[b].T)
        m["posT"] = posT
        m["wq"] = _bf16(inputs["Wq"][:, :, dsl])
        m["wk"] = _bf16(inputs["Wk"][:, :, dsl])
        m["wv"] = _bf16(inputs["Wv"][:, :, dsl])
        m["wp"] = _bf16(inputs["Wp"][:, :, dsl])
        m["wo"] = _bf16(inputs["Wo"][:, dsl, :])
        m["w1"] = _bf16(inputs["W1"][:, :, fsl])
        m["w2"] = _bf16(inputs["W2"][:, fsl, :])

        def head_arr(base, extra=None):
            a = np.asarray(base, np.float32)[:, dsl].reshape(L, cfg.HB, 2, cfg.D)
            if extra is not None:
                a = a + np.asarray(extra, np.float32)[:, hsl].reshape(
                    L, cfg.HB, 2, cfg.D)
            return np.ascontiguousarray(
                a.transpose(0, 2, 3, 1).reshape(L, 128, cfg.HB))
        m["ub"] = head_arr(inputs["bq"], inputs["pos_bias_u"])
        m["vb"] = head_arr(inputs["bq"], inputs["pos_bias_v"])
        m["kb"] = head_arr(inputs["bk"])
        m["vrow"] = _bf16(np.asarray(inputs["bv"], np.float32)[:, dsl]
                          .reshape(L, 1, cfg.DH))
        m["bo"] = _cvec(inputs["bo"], L, cfg.CCH)
        m["bf2"] = _cvec(inputs["bf2"], L, cfg.CCH)
        m["bf1"] = _cvec(np.asarray(inputs["bf1"], np.float32)[:, fsl], L, cfg.FCH)
        m["g1"] = _cvec(inputs["g1"], L, cfg.CCH)
        m["b1"] = _cvec(inputs["beta1"], L, cfg.CCH)
        m["g2"] = _cvec(inputs["g2"], L, cfg.CCH)
        m["b2"] = _cvec(inputs["beta2"], L, cfg.CCH)
        m["gf"] = _cvec(inputs["gf"][None], 1, cfg.CCH)[0]
        m["bf"] = _cvec(inputs["betaf"][None], 1, cfg.CCH)[0]
        mb = np.where(np.asarray(mask[b], bool), 0.0, NEG_MASK).astype(np.float32)
        m["maskb"] = np.ascontiguousarray(mb.reshape(cfg.KCH, 128).T)
        in_maps.append(m)
    return in_maps


def assemble_out(cfg, res):
    """res: list of per-core result dicts -> full [B, N, C] f32 output."""
    outs = []
    for b in range(cfg.DP):
        o = res[b * cfg.TP]["out"]                     # [128, CCH, N]
        full = o.transpose(1, 0, 2).reshape(cfg.C, cfg.N)
        outs.append(full.T)
    return np.stack(outs).astype(np.float32)


_CACHE = {}


def _get_nc(cfg, apply_gb, use_vrow):
    key = (tuple(sorted(cfg.__dict__.items())), apply_gb, use_vrow)
    if key not in _CACHE:
        _CACHE[key] = build_nc(cfg, apply_gb=apply_gb, use_vrow=use_vrow)
    return _CACHE[key]


def run(cfg, inputs, trace=False):
    apply_gb = not (
        np.allclose(inputs["g1"], 1) and np.allclose(inputs["beta1"], 0)
        and np.allclose(inputs["g2"], 1) and np.allclose(inputs["beta2"], 0)
        and np.allclose(inputs["gf"], 1) and np.allclose(inputs["betaf"], 0))
    use_vrow = not np.allclose(inputs["bv"], 0)
    nc = _get_nc(cfg, apply_gb, use_vrow)
    in_maps = make_in_maps(cfg, inputs)
    r = run_bass_kernel_spmd(nc, in_maps, core_ids=list(range(cfg.ncores)),
                             trace=trace)
    return assemble_out(cfg, r.results), r


def kernel(**inputs) -> np.ndarray:
    cfg = Cfg()
    out, _ = run(cfg, inputs)
    return out


# revision 60
# speedup vs baseline: 1.1092x; 1.1092x over previous
"""Trainium2 Bass kernel for nn_AudioEncoder (4-layer conformer-style encoder,
TransformerXL relative-position attention), distributed over 8 NeuronCores.

Sharding: DP=2 over batch (cores 0-3 <-> batch 0, cores 4-7 <-> batch 1),
TP=4 within each quad over heads (4 heads/core) and the FFN dim (1024/core).
Two AllReduces per layer (attention output proj, FFN second matmul) over each
4-core replica group, in bf16; each AllReduce is split into token-halves that
pipeline against the other half's compute so the TensorEngine never idles
across a collective.

On-device layout is feature-major: activations live as [C, tokens] so every
matmul contracts over the partition axis. LayerNorm statistics (reductions
over C = partitions) are computed with [128,128]-ones matmuls that land the
sums pre-broadcast on all partitions (single-partition DVE ops are ~9ns/elem
and must be avoided).
"""

import contextlib

import numpy as np
import ml_dtypes

import concourse.bass as bass
import concourse.mybir as mybir
import concourse.tile as tile
from concourse import bacc
from concourse.bass_utils import run_bass_kernel_spmd

BF16 = mybir.dt.bfloat16
F32 = mybir.dt.float32
AF = mybir.ActivationFunctionType
ALU = mybir.AluOpType
NEG_MASK = -30000.0


class Cfg:
    def __init__(self, L=4, B=2, N=1024, C=1024, H=16, D=64, F=4096,
                 TP=4, eps=1e-5, nsp=None):
        self.L, self.B, self.N, self.C, self.H, self.D, self.F = \
            L, B, N, C, H, D, F
        self.TP = TP
        self.DP = 2
        self.ncores = self.TP * self.DP
        self.eps = eps
        self.HL = H // TP            # heads per core
        self.DH = self.HL * D        # head dims per core
        self.FL = F // TP            # ffn dims per core
        self.CCH = C // 128          # C chunks
        self.HB = self.DH // 128     # head blocks (2 heads each)
        self.FCH = self.FL // 128    # ffn chunks
        self.KCH = N // 128          # token chunks
        self.NSP = nsp or min(512, N)  # matmul free-dim / token group size
        self.NH = N // self.NSP      # number of token groups
        self.scale = D ** -0.5
        assert self.HL % 2 == 0 and D == 64 and C % 128 == 0
        assert self.FL % 128 == 0 and N % 128 == 0


def build_nc(cfg, apply_gb=True, use_vrow=True):
    """Builds the per-core Bass program (SPMD: same graph on all 8 cores)."""
    nc = bacc.Bacc("TRN2", target_bir_lowering=False, debug=False,
                   num_devices=cfg.ncores)
    L, N, CCH, HB, FCH, KCH, NSP = (cfg.L, cfg.N, cfg.CCH, cfg.HB, cfg.FCH,
                                    cfg.KCH, cfg.NSP)
    NHS = [slice(i * NSP, (i + 1) * NSP) for i in range(cfg.NH)]
    G = cfg.NH
    RG = [[g * cfg.TP + i for i in range(cfg.TP)] for g in range(cfg.DP)]

    # ---- DRAM I/O ----
    d = {}
    d["xT"] = nc.dram_tensor("xT", [cfg.C, N], BF16, kind="ExternalInput").ap()
    d["posT"] = nc.dram_tensor("posT", [cfg.C, N], BF16, kind="ExternalInput").ap()
    for nm, shp in [("wq", [L, cfg.C, cfg.DH]), ("wk", [L, cfg.C, cfg.DH]),
                    ("wv", [L, cfg.C, cfg.DH]), ("wp", [L, cfg.C, cfg.DH]),
                    ("wo", [L, cfg.DH, cfg.C]), ("w1", [L, cfg.C, cfg.FL]),
                    ("w2", [L, cfg.FL, cfg.C])]:
        d[nm] = nc.dram_tensor(nm, shp, BF16, kind="ExternalInput").ap()
    for nm, shp in [("ub", [L, 128, HB]), ("vb", [L, 128, HB]),
                    ("kb", [L, 128, HB]),
                    ("bo", [L, 128, CCH]), ("bf2", [L, 128, CCH]),
                    ("bf1", [L, 128, FCH]),
                    ("g1", [L, 128, CCH]), ("b1", [L, 128, CCH]),
                    ("g2", [L, 128, CCH]), ("b2", [L, 128, CCH]),
                    ("gf", [128, CCH]), ("bf", [128, CCH]),
                    ("maskb", [128, KCH])]:
        d[nm] = nc.dram_tensor(nm, shp, F32, kind="ExternalInput").ap()
    d["vrow"] = nc.dram_tensor("vrow", [L, 1, cfg.DH], BF16,
                               kind="ExternalInput").ap()
    out_d = nc.dram_tensor("out", [128, CCH, N], F32, kind="ExternalOutput").ap()

    ones_bc_d = nc.inline_tensor(np.ones((128, 128), ml_dtypes.bfloat16), "ones_bc")
    ones64_d = nc.inline_tensor(np.ones((128, 64), ml_dtypes.bfloat16), "ones64")
    ones_row_d = nc.inline_tensor(np.ones((1, 128), ml_dtypes.bfloat16), "ones_row")

    with tile.TileContext(nc) as tc:
        ctx = contextlib.ExitStack()
        with ctx:
            consts = ctx.enter_context(tc.tile_pool(name="consts", bufs=1))
            resid = ctx.enter_context(tc.tile_pool(name="resid", bufs=1))
            wpool = ctx.enter_context(tc.tile_pool(name="wpool", bufs=1))
            work = ctx.enter_context(tc.tile_pool(name="work", bufs=1))
            lnw = ctx.enter_context(tc.tile_pool(name="lnw", bufs=2))
            tmp = ctx.enter_context(tc.tile_pool(name="tmp", bufs=4))
            strm = ctx.enter_context(tc.tile_pool(name="strm", bufs=3))
            psum = ctx.enter_context(tc.tile_pool(name="psum", bufs=2, space="PSUM"))
            sumsp = ctx.enter_context(tc.tile_pool(name="sumsp", bufs=2, space="PSUM"))
            dram = ctx.enter_context(tc.tile_pool(name="dram", bufs=2, space="DRAM"))

            # ---- constants / resident tensors ----
            ones_bc = consts.tile([128, 128], BF16)
            nc.sync.dma_start(ones_bc, ones_bc_d.ap())
            ones_row = consts.tile([1, 128], BF16)
            nc.sync.dma_start(ones_row, ones_row_d.ap())
            ones64 = consts.tile([128, 64], BF16)
            nc.sync.dma_start(ones64, ones64_d.ap())
            gf_t = consts.tile([128, CCH], F32)
            nc.sync.dma_start(gf_t, d["gf"])
            bfin_t = consts.tile([128, CCH], F32)
            nc.sync.dma_start(bfin_t, d["bf"])
            maskb_t = consts.tile([128, KCH], F32)
            nc.sync.dma_start(maskb_t, d["maskb"])
            eps_t = consts.tile([128, 1], F32)
            nc.vector.memset(eps_t, float(cfg.eps))
            zero_t = consts.tile([128, 1], F32)
            nc.vector.memset(zero_t, 0.0)
            posT = consts.tile([128, CCH, N], BF16)
            nc.sync.dma_start(posT, d["posT"].rearrange("(k p) n -> p k n", p=128))

            hT = resid.tile([128, CCH, N], BF16)
            xr = d["xT"].rearrange("(k p) n -> p k n", p=128)
            for i in range(cfg.NH):   # per-group loads so LN1(g0) starts early
                nhg = slice(i * NSP, (i + 1) * NSP)
                nc.sync.dma_start(hT[:, :, nhg], xr[:, :, nhg])

            def _load(l, mats, smalls, vrow=False):
                w = {}
                for nm, shp in mats:
                    w[nm] = wpool.tile(shp, BF16, tag=nm, name=nm)
                    nc.sync.dma_start(
                        w[nm], d[nm][l].rearrange("(k p) m -> p k m", p=128))
                for nm, width in smalls:
                    w[nm] = wpool.tile([128, width], F32, tag=nm,
                                       name=f"sm_{nm}")
                    nc.sync.dma_start(w[nm], d[nm][l])
                if vrow and use_vrow:
                    w["vrow"] = wpool.tile([1, cfg.DH], BF16, tag="vrow",
                                           name="vrow")
                    nc.sync.dma_start(w["vrow"], d["vrow"][l])
                return w

            def load_w_qkvp(l):
                # weights consumed by LN1(l)/qkv(l) (emitted in layer l-1's
                # tail) and p-proj(l) — safe to load during layer l-1.
                return _load(
                    l,
                    [("wq", [128, CCH, cfg.DH]), ("wk", [128, CCH, cfg.DH]),
                     ("wv", [128, CCH, cfg.DH]), ("wp", [128, CCH, cfg.DH])],
                    [("ub", HB), ("vb", HB), ("kb", HB), ("g1", CCH),
                     ("b1", CCH)],
                    vrow=True)

            def load_w_rest(l):
                return _load(
                    l,
                    [("wo", [128, HB, cfg.C]), ("w1", [128, CCH, cfg.FL]),
                     ("w2", [128, FCH, cfg.C])],
                    [("bo", CCH), ("bf2", CCH), ("bf1", FCH), ("g2", CCH),
                     ("b2", CCH)])

            # ---------- per-token-group layernorm (feature-major) ----------
            def ln_stats(src, nhg):
                """LN stats over C (partitions) for token group nhg. The
                [128,128] ones lhsT lands sums pre-broadcast on all
                partitions. Returns bf16 (a, b) rows: hn = x*a + b."""
                s0 = sumsp.tile([128, NSP], F32, tag="asum", bufs=2, name="s0")
                s1 = sumsp.tile([128, NSP], F32, tag="asum", bufs=2, name="s1")
                for c in range(CCH):
                    xsq = tmp.tile([128, NSP], BF16, tag="tmp")
                    nc.vector.tensor_mul(xsq, src[:, c, nhg], src[:, c, nhg])
                    nc.tensor.matmul(s0, ones_bc, src[:, c, nhg],
                                     start=(c == 0), stop=(c == CCH - 1))
                    nc.tensor.matmul(s1, ones_bc, xsq,
                                     start=(c == 0), stop=(c == CCH - 1))
                # msq = (s0/C)^2 ; var = s1/C - msq ; a = 1/sqrt(var+eps)
                t_sb = lnw.tile([128, NSP], F32, tag="t_sb")
                nc.scalar.activation(t_sb, s0, AF.Square, bias=zero_t,
                                     scale=1.0 / cfg.C)
                nc.vector.scalar_tensor_tensor(
                    t_sb, in0=s1, scalar=1.0 / cfg.C, in1=t_sb,
                    op0=ALU.mult, op1=ALU.subtract)
                nc.scalar.activation(t_sb, t_sb, AF.Sqrt, bias=eps_t)
                a_sb = lnw.tile([128, NSP], F32, tag="a_sb")
                nc.vector.reciprocal_approx_fast(a_sb, t_sb)
                # bf16 scale/shift rows: all-bf16 SBUF operands run the DVE
                # in 4x mode for the apply passes; b = -(s0/C)*a
                a_bf = lnw.tile([128, NSP], BF16, tag="a_bf")
                nc.vector.tensor_copy(a_bf, a_sb)
                b_bf = lnw.tile([128, NSP], BF16, tag="b_bf")
                nc.vector.scalar_tensor_tensor(
                    b_bf, in0=s0, scalar=-1.0 / cfg.C, in1=a_sb,
                    op0=ALU.mult, op1=ALU.mult)
                return a_bf, b_bf

            def ln_apply(src, nhg, ab, g_col, beta_col, dst=None,
                         out_f32=False):
                a_bf, b_bf = ab
                for c in range(CCH):
                    t1 = tmp.tile([128, NSP], BF16, tag="tmp")
                    nc.vector.tensor_mul(t1, src[:, c, nhg], a_bf)
                    if out_f32:
                        oc = strm.tile([128, NSP], F32, tag="outc")
                        nc.vector.tensor_add(oc, t1, b_bf)
                        if apply_gb:
                            nc.vector.tensor_scalar(
                                oc, oc, g_col[:, c:c + 1], beta_col[:, c:c + 1],
                                op0=ALU.mult, op1=ALU.add)
                        nc.sync.dma_start(out_d[:, c, nhg], oc)
                    elif apply_gb:
                        t2 = tmp.tile([128, NSP], BF16, tag="tmp")
                        nc.vector.tensor_add(t2, t1, b_bf)
                        nc.vector.tensor_scalar(
                            dst[:, c, nhg], t2, g_col[:, c:c + 1],
                            beta_col[:, c:c + 1], op0=ALU.mult, op1=ALU.add)
                    else:
                        nc.vector.tensor_add(dst[:, c, nhg], t1, b_bf)

            # ---------- q/k/v projections for one token group ----------
            def emit_qkv(w, hn, qTu, qTv, kT, v_t, g):
                nhg = NHS[g]
                for blk in range(HB):
                    ps = psum.tile([128, NSP], F32, tag="mm")
                    for c in range(CCH):
                        nc.tensor.matmul(
                            ps, w["wq"][:, c, blk * 128:(blk + 1) * 128],
                            hn[:, c, nhg], start=(c == 0), stop=(c == CCH - 1))
                    nc.vector.tensor_scalar(qTu[:, blk, nhg], ps,
                                            w["ub"][:, blk:blk + 1], None,
                                            op0=ALU.add)
                    nc.vector.tensor_scalar(qTv[:, blk, nhg], ps,
                                            w["vb"][:, blk:blk + 1], None,
                                            op0=ALU.add)
                    ps = psum.tile([128, NSP], F32, tag="mm")
                    for c in range(CCH):
                        nc.tensor.matmul(
                            ps, w["wk"][:, c, blk * 128:(blk + 1) * 128],
                            hn[:, c, nhg], start=(c == 0), stop=(c == CCH - 1))
                    nc.vector.tensor_scalar(kT[:, blk, nhg], ps,
                                            w["kb"][:, blk:blk + 1], None,
                                            op0=ALU.add)
                # v: token-major
                for kc in range(g * KCH // G, (g + 1) * KCH // G):
                    ps = psum.tile([128, cfg.DH], F32, tag="mm")
                    for c in range(CCH):
                        nc.tensor.matmul(
                            ps, hn[:, c, kc * 128:(kc + 1) * 128], w["wv"][:, c, :],
                            start=(c == 0),
                            stop=(c == CCH - 1) and not use_vrow)
                    if use_vrow:
                        nc.tensor.matmul(ps, ones_row, w["vrow"],
                                         start=False, stop=True)
                    nc.vector.tensor_copy(v_t[:, kc, :], ps)

            def emit_pproj(wA, l):
                pT = work.tile([128, HB, N], BF16, tag="pT", name=f"pT{l}",
                               bufs=2)
                for blk in range(HB):
                    for nh in NHS:
                        ps = psum.tile([128, NSP], F32, tag="mm")
                        for c in range(CCH):
                            nc.tensor.matmul(
                                ps, wA["wp"][:, c, blk * 128:(blk + 1) * 128],
                                posT[:, c, nh],
                                start=(c == 0), stop=(c == CCH - 1))
                        nc.vector.tensor_copy(pT[:, blk, nh], ps)
                return pT

            # ================= program =================
            wA = load_w_qkvp(0)
            hn = work.tile([128, CCH, N], BF16, tag="hn", name="hn0", bufs=2)
            qTu = work.tile([128, HB, N], BF16, tag="qTu", name="qTu0")
            qTv = work.tile([128, HB, N], BF16, tag="qTv", name="qTv0")
            kT = work.tile([128, HB, N], BF16, tag="kT", name="kT0")
            v_t = work.tile([128, KCH, cfg.DH], BF16, tag="v_t", name="v0")
            for g in range(G):
                ab = ln_stats(hT, NHS[g])
                ln_apply(hT, NHS[g], ab, wA["g1"], wA["b1"], dst=hn)
                emit_qkv(wA, hn, qTu, qTv, kT, v_t, g)
            pT = emit_pproj(wA, 0)

            def emit_resid(g, ar_out, bias_col):
                nhg = NHS[g]
                for cc in range(CCH):
                    art = strm.tile([128, NSP], BF16, tag="art")
                    nc.sync.dma_start(art, ar_out[:, cc, :])
                    nc.vector.scalar_tensor_tensor(
                        hT[:, cc, nhg], in0=art, scalar=bias_col[:, cc:cc + 1],
                        in1=hT[:, cc, nhg], op0=ALU.add, op1=ALU.add)

            def emit_allreduce(src_mm, g, name):
                """src_mm(cc) emits accumulating matmuls into a psum tile;
                copies to bf16, DMAs to a DRAM bounce, AllReduces."""
                ai = dram.tile([128, CCH, NSP], BF16, tag="ar_in",
                               name=f"{name}i")
                ao = dram.tile([128, CCH, NSP], BF16, tag="ar_out",
                               name=f"{name}o")
                for cc in range(CCH):
                    ps = psum.tile([128, NSP], F32, tag="mm")
                    src_mm(ps, cc)
                    arc = strm.tile([128, NSP], BF16, tag="arc")
                    nc.vector.tensor_copy(arc, ps)
                    nc.sync.dma_start(ai[:, cc, :], arc)
                nc.gpsimd.collective_compute(
                    "AllReduce", ALU.add, replica_groups=RG,
                    ins=[ai[:].opt()], outs=[ao[:].opt()])
                return ao

            for l in range(L):
                w = {**wA, **load_w_rest(l)}
                last = l == L - 1

                # -- attention + output projection + AR1, token-group-outer:
                # group g0's AllReduce launches while group g1's attention
                # still computes, so the collective hides behind the PE.
                oTn = work.tile([128, HB, N], BF16, tag="oTn", name=f"oTn{l}")
                ar1_out = []
                ab2 = {}
                for ig, nh in enumerate(NHS):
                    po = {}
                    sep = {}
                    for blk in range(HB):
                        po[blk] = psum.tile([128, NSP], F32, tag="po",
                                            bufs=2, name=f"po{blk}")
                        sep[blk] = sumsp.tile([128, NSP], F32, tag="sep",
                                              bufs=2, name=f"sep{blk}")
                    for kc in range(KCH):
                        kcs = slice(kc * 128, (kc + 1) * 128)
                        for blk in range(HB):
                            es = {}
                            # score pairs share PE row groups (K=64, rows 0/64)
                            for hr in (0, 1):
                                psl = slice(hr * 64, hr * 64 + 64)
                                s = psum.tile([128, NSP], F32, tag="mm",
                                              name=f"s{hr}")
                                nc.tensor.matmul(s, kT[psl, blk, kcs],
                                                 qTu[psl, blk, nh],
                                                 start=True, stop=False)
                                nc.tensor.matmul(s, pT[psl, blk, kcs],
                                                 qTv[psl, blk, nh],
                                                 start=False, stop=True)
                                es[hr] = s
                            ee = {}
                            for hr in (0, 1):
                                e = tmp.tile([128, NSP], BF16, tag="tmp",
                                             name=f"e{hr}")
                                nc.scalar.activation(e, es[hr], AF.Exp,
                                                     bias=maskb_t[:, kc:kc + 1],
                                                     scale=float(cfg.scale))
                                ee[hr] = e
                            # attn@v and exp-sum pairs share PE column groups
                            for hr in (0, 1):
                                h = blk * 2 + hr
                                psl = slice(hr * 64, hr * 64 + 64)
                                nc.tensor.matmul(
                                    po[blk][psl, :],
                                    v_t[:, kc, h * 64:(h + 1) * 64],
                                    ee[hr], start=(kc == 0),
                                    stop=(kc == KCH - 1), skip_group_check=True)
                            for hr in (0, 1):
                                psl = slice(hr * 64, hr * 64 + 64)
                                nc.tensor.matmul(
                                    sep[blk][psl, :], ones64, ee[hr],
                                    start=(kc == 0), stop=(kc == KCH - 1),
                                    skip_group_check=True)
                    for blk in range(HB):
                        sesb = lnw.tile([128, NSP], F32, tag="sesb")
                        nc.vector.tensor_copy(sesb, sep[blk])
                        rcp = lnw.tile([128, NSP], F32, tag="rcpf")
                        nc.vector.reciprocal_approx_fast(rcp, sesb)
                        nc.vector.tensor_mul(oTn[:, blk, nh], po[blk], rcp)

                    def oproj_mm(ps, cc, nh=nh):
                        for db in range(HB):
                            nc.tensor.matmul(
                                ps, w["wo"][:, db, cc * 128:(cc + 1) * 128],
                                oTn[:, db, nh],
                                start=(db == 0), stop=(db == HB - 1))
                    ar1_out.append(emit_allreduce(oproj_mm, ig, f"a1_{ig}"))
                    if ig == 0 and not last:
                        # weight DMAs issue early; p-proj is PE filler
                        wA_next = load_w_qkvp(l + 1)
                        pT_next = emit_pproj(wA_next, l + 1)

                # -- FFN pipeline: group g's FFN runs under the other
                # group's AllReduces; resid+stats hoisted to overlap PE work
                hn2 = work.tile([128, CCH, N], BF16, tag="hn",
                                name=f"hn2_{l}", bufs=2)
                fT = work.tile([128, FCH, N], BF16, tag="fT", name=f"fT{l}")
                ar2_out = {}

                def emit_ffn(g):
                    nhg = NHS[g]
                    ln_apply(hT, nhg, ab2[g], w["g2"], w["b2"], dst=hn2)
                    for fc in range(FCH):
                        ps = psum.tile([128, NSP], F32, tag="mm")
                        for c in range(CCH):
                            nc.tensor.matmul(
                                ps, w["w1"][:, c, fc * 128:(fc + 1) * 128],
                                hn2[:, c, nhg],
                                start=(c == 0), stop=(c == CCH - 1))
                        nc.scalar.activation(fT[:, fc, nhg], ps, AF.Relu,
                                             bias=w["bf1"][:, fc:fc + 1])

                    def ffn2_mm(ps, cc, nhg=nhg):
                        for fc in range(FCH):
                            nc.tensor.matmul(
                                ps, w["w2"][:, fc, cc * 128:(cc + 1) * 128],
                                fT[:, fc, nhg],
                                start=(fc == 0), stop=(fc == FCH - 1))
                    ar2_out[g] = emit_allreduce(ffn2_mm, g, f"a2_{g}")

                for g in range(G):
                    emit_resid(g, ar1_out[g], w["bo"])
                    ab2[g] = ln_stats(hT, NHS[g])
                    emit_ffn(g)

                # -- tail: residual2 + (next-layer LN1+qkv | final LN) --
                if not last:
                    hn = work.tile([128, CCH, N], BF16, tag="hn",
                                   name=f"hn{l + 1}", bufs=2)
                    qTu = work.tile([128, HB, N], BF16, tag="qTu",
                                    name=f"qTu{l + 1}")
                    qTv = work.tile([128, HB, N], BF16, tag="qTv",
                                    name=f"qTv{l + 1}")
                    kT = work.tile([128, HB, N], BF16, tag="kT",
                                   name=f"kT{l + 1}")
                    v_t = work.tile([128, KCH, cfg.DH], BF16, tag="v_t",
                                    name=f"v{l + 1}")
                for g in range(G):
                    emit_resid(g, ar2_out[g], w["bf2"])
                    ab = ln_stats(hT, NHS[g])
                    if last:
                        ln_apply(hT, NHS[g], ab, gf_t, bfin_t, out_f32=True)
                    else:
                        ln_apply(hT, NHS[g], ab, wA_next["g1"],
                                 wA_next["b1"], dst=hn)
                        emit_qkv(wA_next, hn, qTu, qTv, kT, v_t, g)
                if not last:
                    wA = wA_next
                    pT = pT_next

    nc.compile()
    return nc


# ---------------------------------------------------------------------------
# host side
# ---------------------------------------------------------------------------

def _bf16(a):
    return np.ascontiguousarray(np.asarray(a, np.float32).astype(ml_dtypes.bfloat16))


def _cvec(a, L, nch):
    # [L, nch*128] -> [L, 128, nch]
    return np.ascontiguousarray(
        np.asarray(a, np.float32).reshape(L, nch, 128).transpose(0, 2, 1))


def make_in_maps(cfg, inputs):
    """Shard + lay out full inputs for the 8 cores."""
    L, TP = cfg.L, cfg.TP
    x = np.asarray(inputs["x"], np.float32)
    mask = np.asarray(inputs["attention_mask"])
    pos = np.asarray(inputs["pos_embeds"], np.float32)[0]      # [N, C]
    posT = _bf16(pos.T)
    in_maps = []
    for core in range(cfg.ncores):
        tp, b = core % TP, core // TP
        dsl = slice(tp * cfg.DH, (tp + 1) * cfg.DH)
        hsl = slice(tp * cfg.HL, (tp + 1) * cfg.HL)
        fsl = slice(tp * cfg.FL, (tp + 1) * cfg.FL)
        m = {}
        m["xT"] = _bf16(x[b].T)
        m["posT"] = posT
        m["wq"] = _bf16(inputs["Wq"][:, :, dsl])
        m["wk"] = _bf16(inputs["Wk"][:, :, dsl])
        m["wv"] = _bf16(inputs["Wv"][:, :, dsl])
        m["wp"] = _bf16(inputs["Wp"][:, :, dsl])
        m["wo"] = _bf16(inputs["Wo"][:, dsl, :])
        m["w1"] = _bf16(inputs["W1"][:, :, fsl])
        m["w2"] = _bf16(inputs["W2"][:, fsl, :])

        def head_arr(base, extra=None):
            a = np.asarray(base, np.float32)[:, dsl].reshape(L, cfg.HB, 2, cfg.D)
            if extra is not None:
                a = a + np.asarray(extra, np.float32)[:, hsl].reshape(
                    L, cfg.HB, 2, cfg.D)
            return np.ascontiguousarray(
                a.transpose(0, 2, 3, 1).reshape(L, 128, cfg.HB))
        m["ub"] = head_arr(inputs["bq"], inputs["pos_bias_u"])
        m["vb"] = head_arr(inputs["bq"], inputs["pos_bias_v"])
        m["kb"] = head_arr(inputs["bk"])
        m["vrow"] = _bf16(np.asarray(inputs["bv"], np.float32)[:, dsl]
                          .reshape(L, 1, cfg.DH))
        m["bo"] = _cvec(inputs["bo"], L, cfg.CCH)
        m["bf2"] = _cvec(inputs["bf2"], L, cfg.CCH)
        m["bf1"] = _cvec(np.asarray(inputs["bf1"], np.float32)[:, fsl], L, cfg.FCH)
        m["g1"] = _cvec(inputs["g1"], L, cfg.CCH)
        m["b1"] = _cvec(inputs["beta1"], L, cfg.CCH)
        m["g2"] = _cvec(inputs["g2"], L, cfg.CCH)
        m["b2"] = _cvec(inputs["beta2"], L, cfg.CCH)
        m["gf"] = _cvec(inputs["gf"][None], 1, cfg.CCH)[0]
        m["bf"] = _cvec(inputs["betaf"][None], 1, cfg.CCH)[0]
        mb = np.where(np.asarray(mask[b], bool), 0.0, NEG_MASK).astype(np.float32)
        m["maskb"] = np.ascontiguousarray(mb.reshape(cfg.KCH, 128).T)
        in_maps.append(m)
    return in_maps


def assemble_out(cfg, res):
    """res: list of per-core result dicts -> full [B, N, C] f32 output."""
    outs = []
    for b in range(cfg.DP):
        o = res[b * cfg.TP]["out"]                     # [128, CCH, N]
        full = o.transpose(1, 0, 2).reshape(cfg.C, cfg.N)
        outs.append(full.T)
    return np.stack(outs).astype(np.float32)


_CACHE = {}


def _get_nc(cfg, apply_gb, use_vrow):
    key = (tuple(sorted(cfg.__dict__.items())), apply_gb, use_vrow)
    if key not in _CACHE:
        _CACHE[key] = build_nc(cfg, apply_gb=apply_gb, use_vrow=use_vrow)
    return _CACHE[key]


def run(cfg, inputs, trace=False):
    apply_gb = not (
        np.allclose(inputs["g1"], 1) and np.allclose(inputs["beta1"], 0)
        and np.allclose(inputs["g2"], 1) and np.allclose(inputs["beta2"], 0)
        and np.allclose(inputs["gf"], 1) and np.allclose(inputs["betaf"], 0))
    use_vrow = not np.allclose(inputs["bv"], 0)
    nc = _get_nc(cfg, apply_gb, use_vrow)
    in_maps = make_in_maps(cfg, inputs)
    r = run_bass_kernel_spmd(nc, in_maps, core_ids=list(range(cfg.ncores)),
                             trace=trace)
    return assemble_out(cfg, r.results), r


def kernel(**inputs) -> np.ndarray:
    cfg = Cfg()
    out, _ = run(cfg, inputs)
    return out
